# revision 32
# baseline (speedup 1.0000x reference)
"""Trainium2 Bass kernel for nn_Critic (gnn_message_passing) — v4.

Strategy (8 NeuronCores, one SPMD NEFF):
  Phase 1 (node-sharded, 8 nodes/core): per-node MLPs fully in bf16 on PE.
    LN rsqrt = ACT Sqrt(var+eps) + DVE reciprocal; scale/bias/relu fused
    into one ACT Relu(g*x+be) per chunk (every LUT function used lives in
    the single 'sqrt_and_others' table -> one table load total). The mm2
    biases ride for free on the existing ACT copy (bias=b2V) and the DVE
    q=A+V op (scalar_tensor_tensor +b2A). Per node we emit the d-sums of
    Q and V (one ones-matmul) — the Choquet singles+center source — in
    bf16, and an fp8 copy of Q,V for the collective.
  Phase 2: AllGather of the fp8 Q/V block (half the bytes of bf16), then
    a second tiny AllGather of the bf16 d-sums. fp8 only touches the pair
    (min) terms whose error is ~10x below the center term carried by the
    bf16 sums.
  Phase 3 (set-sharded): X gathered as fp8 64-row blocks via
    register-offset HWDGE DMAs (offsets preloaded on both queues during
    the collective), upconverted fp8->bf16 on the ACT engine so the DVE
    pairmin keeps its 2x 16-bit rate; weighted d+pair reduction on PE via
    diagonal-block matmuls [4, 4B]; singles+center ride as one extra
    accumulated matmul per (set,t) whose off-diagonal garbage lands in
    fold-discarded entries; fold + output stream out per half.
"""

import numpy as np
import ml_dtypes

import concourse.bass as bass
import concourse.bacc as bacc
import concourse.mybir as mybir
from concourse import tile
from concourse.bass_utils import run_bass_kernel_spmd

B, N, H, D, K, HEADS = 128, 64, 256, 128, 8, 3
NCORE = 8
NLOC = N // NCORE      # nodes per core
SLOC = N // NCORE      # sets per core
NINST = 2 * SLOC       # (set, t) instances per core
NSLOT = K + 1          # center + 8 neighbors
NPAIR = (K * (K - 1)) // 2  # 28
CH_ROWS = 4 * D       # rows per fp8 AllGather chunk (4 nodes)
F32 = mybir.dt.float32
BF16 = mybir.dt.bfloat16
F8 = mybir.dt.float8e4
I32 = mybir.dt.int32

# pairs in delta-major order over neighbor slots 1..8
PAIRS = [(a, a + d) for d in range(1, K) for a in range(1, K - d + 1)]

_compiled = None


def _build():
    nc = bacc.Bacc("TRN2", target_bir_lowering=False, debug=False,
                   num_devices=NCORE)

    # ---- per-core inputs (host-packed) ----
    xin = nc.dram_tensor("xin", [NLOC, 128, 4, B], BF16, kind="ExternalInput")
    wp = nc.dram_tensor("wp", [NLOC, 128, 2048], BF16, kind="ExternalInput")
    bia = nc.dram_tensor("bia", [1, NLOC * 512], BF16, kind="ExternalInput")
    b2p = nc.dram_tensor("b2p", [128, NLOC, 2], F32, kind="ExternalInput")
    lnw = nc.dram_tensor("lnw", [128, 8], F32, kind="ExternalInput")
    pw = nc.dram_tensor("pw", [128, SLOC, 7, 4], BF16, kind="ExternalInput")
    ws = nc.dram_tensor("ws", [64, SLOC, 4], BF16, kind="ExternalInput")
    gb = nc.dram_tensor("gb", [1, SLOC * NSLOT], I32, kind="ExternalInput")
    ident = nc.dram_tensor("ident", [128, 128], BF16, kind="ExternalInput")

    chi = nc.dram_tensor("chi", [1, NINST * B], F32, kind="ExternalOutput")

    with tile.TileContext(nc, num_cores=NCORE) as tc:
        with tc.tile_pool(name="const", bufs=1) as cpool, \
             tc.tile_pool(name="dram", bufs=1, space="DRAM") as dram:
            ident_s = cpool.tile([128, 128], BF16)
            nc.sync.dma_start(out=ident_s[:], in_=ident[:])
            ones_row = cpool.tile([1, 128], BF16)
            nc.vector.memset(ones_row[:], 1.0)
            ones_col = cpool.tile([128, 1], BF16)
            nc.vector.memset(ones_col[:], 1.0)
            eps_t = cpool.tile([B, 1], F32)
            nc.vector.memset(eps_t[:], 1e-5)
            lnw_s = cpool.tile([128, 8], F32)
            nc.sync.dma_start(out=lnw_s[:], in_=lnw[:])
            b2p_s = cpool.tile([128, NLOC, 2], F32)
            nc.sync.dma_start(out=b2p_s[:], in_=b2p[:])
            pw_s = cpool.tile([128, SLOC, 7, 4], BF16)
            nc.scalar.dma_start(out=pw_s[:], in_=pw[:])
            ws_s = cpool.tile([64, SLOC, 4], BF16)
            nc.scalar.dma_start(out=ws_s[:], in_=ws[:])
            bia_s = cpool.tile([1, NLOC * 512], BF16)
            nc.gpsimd.dma_start(out=bia_s[:], in_=bia[:])
            gb_s = cpool.tile([1, SLOC * NSLOT], I32)
            nc.sync.dma_start(out=gb_s[:], in_=gb[:])
            warm_rhs = cpool.tile([128, 512], BF16)
            nc.vector.memset(warm_rhs[:], 0.0)

            # fp8 gathered q/v: [core, chunk, row=node%4*128+d, (t b)]
            qvloc8 = dram.tile([2, CH_ROWS, 2 * B], F8)
            qvall8 = dram.tile([NCORE, 2, CH_ROWS, 2 * B], F8,
                               addr_space="Shared")
            qv_flat = qvall8.rearrange("c h r w -> (c h r) w")
            # bf16 per-node d-sums of q and v
            qvs_loc = dram.tile([NLOC, 2 * B], BF16)
            qvs_all = dram.tile([NCORE, NLOC, 2 * B], BF16,
                                addr_space="Shared")

            # HAM warm-up: lift the PE clock gate before real matmuls.
            with tc.tile_pool(name="ps_w", bufs=1, space="PSUM") as ps_w:
                wpp = ps_w.tile([128, 512], F32)
                for k in range(10):
                    nc.tensor.matmul(wpp[:], ident_s[:], warm_rhs[:],
                                     start=(k == 0), stop=(k == 9))

            # ================= Phase 1: per-node MLPs =================
            with tc.tile_pool(name="p1", bufs=4) as p1, \
                 tc.tile_pool(name="p1w", bufs=3) as p1w, \
                 tc.tile_pool(name="p1s", bufs=1) as p1s, \
                 tc.tile_pool(name="ps_h", bufs=3, space="PSUM") as ps_h, \
                 tc.tile_pool(name="ps_t", bufs=2, space="PSUM") as ps_t, \
                 tc.tile_pool(name="ps_o", bufs=2, space="PSUM") as ps_o, \
                 tc.tile_pool(name="ps_s", bufs=1, space="PSUM") as ps_s:

                sums8 = p1s.tile([1, NLOC * 2 * B], BF16)

                for i in range(NLOC):
                    xt = p1.tile([128, 4, B], BF16, tag="xt")
                    nc.sync.dma_start(out=xt[:], in_=xin[i])
                    wt = p1w.tile([128, 2048], BF16, tag="wt")
                    nc.scalar.dma_start(out=wt[:], in_=wp[i])
                    bo = i * 512

                    # mm1 for both branches: h[b, (br, o)] in PSUM f32;
                    # both b1 biases in one 512-col matmul
                    h = ps_h.tile([B, 2, H], F32, tag="h")
                    for c in range(2):
                        nc.tensor.matmul(h[:, 0, :], xt[:, c, :],
                                         wt[:, c * 256:(c + 1) * 256],
                                         start=(c == 0), stop=False)
                    nc.tensor.matmul(h[:, 0, :], ones_row[:],
                                     bia_s[:, bo:bo + 256],
                                     start=False, stop=True)
                    for c in range(4):
                        nc.tensor.matmul(h[:, 1, :], xt[:, c, :],
                                         wt[:, 512 + c * 256:768 + c * 256],
                                         start=(c == 0), stop=False)
                    nc.tensor.matmul(h[:, 1, :], ones_row[:],
                                     bia_s[:, bo + 256:bo + 512],
                                     start=False, stop=True)

                    # LN + relu per branch, then mm2
                    ut = ps_t.tile([128, 2, 2, 128], BF16, tag="ut")
                    hT = p1.tile([128, 2, 2, 128], BF16, tag="hT")
                    o = ps_o.tile([D, 2, B], F32, tag="o")
                    for br in range(2):
                        bn6 = p1.tile([B, 6], F32, tag="bn6")
                        nc.vector.bn_stats(bn6[:], h[:, br, :])
                        bn2 = p1.tile([B, 2], F32, tag="bn2")
                        nc.vector.bn_aggr(bn2[:], bn6[:])
                        sd = p1.tile([B, 1], F32, tag="sd")
                        nc.scalar.activation(
                            sd[:], bn2[:, 1:2],
                            mybir.ActivationFunctionType.Sqrt,
                            bias=eps_t[:])
                        rs = p1.tile([B, 1], F32, tag="rs")
                        nc.vector.reciprocal(rs[:], sd[:])
                        # u = (h - mu) * rs
                        u = p1.tile([B, H], BF16, tag="u")
                        nc.vector.tensor_scalar(u[:], h[:, br, :],
                                                bn2[:, 0:1], rs[:],
                                                mybir.AluOpType.subtract,
                                                mybir.AluOpType.mult)
                        for c in range(2):
                            nc.tensor.transpose(ut[:, br, c, :],
                                                u[:, c * 128:(c + 1) * 128],
                                                ident_s[:])
                            # hT = relu(g * ut + be), one ACT op
                            nc.scalar.activation(
                                hT[:, br, c, :], ut[:, br, c, :],
                                mybir.ActivationFunctionType.Relu,
                                scale=lnw_s[:, 4 * br + c:4 * br + c + 1],
                                bias=lnw_s[:, 4 * br + 2 + c:4 * br + 3 + c])
                        w2o = 1536 + br * 256
                        for c in range(2):
                            nc.tensor.matmul(
                                o[:, br, :],
                                wt[:, w2o + c * 128:w2o + (c + 1) * 128],
                                hT[:, br, c, :],
                                start=(c == 0), stop=(c == 1))

                    qvb = p1.tile([D, 2, B], BF16, tag="qvb")
                    # v = oV + b2V (ACT copy with per-partition bias)
                    nc.scalar.activation(
                        qvb[:, 1, :], o[:, 0, :],
                        mybir.ActivationFunctionType.Identity,
                        bias=b2p_s[:, i, 0:1])
                    # q = (oA + b2A) + v  (DVE scalar_tensor_tensor)
                    nc.vector.scalar_tensor_tensor(
                        qvb[:, 0, :], o[:, 1, :], b2p_s[:, i, 1:2],
                        qvb[:, 1, :],
                        mybir.AluOpType.add, mybir.AluOpType.add)
                    # fp8 copy for the collective (Pool engine)
                    qv8 = p1.tile([D, 2, B], F8, tag="qv8")
                    nc.gpsimd.tensor_copy(qv8[:], qvb[:])
                    # d-sums of q and v in one ones-matmul
                    sq = ps_s.tile([1, 2 * B], F32, tag="sq")
                    nc.tensor.matmul(sq[:], ones_col[:],
                                     qvb[:].rearrange("p t b -> p (t b)"),
                                     start=True, stop=True)
                    nc.scalar.copy(
                        sums8[:, i * 2 * B:(i + 1) * 2 * B], sq[:])
                    # store node rows into its chunk
                    hh, i4 = (0, i) if i < 4 else (1, i - 4)
                    nc.sync.dma_start(
                        out=qvloc8[hh, i4 * 128:(i4 + 1) * 128, :],
                        in_=qv8[:].rearrange("p t b -> p (t b)"))

                nc.gpsimd.dma_start(out=qvs_loc[:], in_=sums8[:])

            # ================= Phase 2: AllGathers ====================
            nc.gpsimd.collective_compute(
                "AllGather", mybir.AluOpType.bypass,
                replica_groups=[list(range(NCORE))],
                ins=[qvloc8.opt()], outs=[qvall8.opt()],
            )
            nc.gpsimd.collective_compute(
                "AllGather", mybir.AluOpType.bypass,
                replica_groups=[list(range(NCORE))],
                ins=[qvs_loc.opt()], outs=[qvs_all.opt()],
            )

            # ================= Phase 3: Choquet =======================
            SG = 2  # sets per group
            NG = SLOC // SG
            with tc.tile_pool(name="p3x8", bufs=2) as p3x8, \
                 tc.tile_pool(name="p3", bufs=2) as p3, \
                 tc.tile_pool(name="p3pm", bufs=2) as p3pm, \
                 tc.tile_pool(name="p3c", bufs=1) as p3c, \
                 tc.tile_pool(name="ps_p", bufs=3, space="PSUM") as ps_p, \
                 tc.tile_pool(name="ps_w2", bufs=1, space="PSUM") as psw, \
                 tc.tile_pool(name="ps_r", bufs=1, space="PSUM") as ps_r:

                # d-sums for all 64 nodes -> [64, 2, B], replicated x4
                sq_all = p3c.tile([64, 2, B], BF16)
                nc.sync.dma_start(out=sq_all[:], in_=qvs_all[:])
                sq_rep = p3c.tile([64, 2, 4, B], BF16)
                for r in range(4):
                    nc.vector.tensor_copy(sq_rep[:, :, r, :], sq_all[:])

                # preload gather offsets while the collective is in
                # flight; each issuing engine holds only what it uses
                vals_sp, vals_act = [], []
                for g in range(NG):
                    j0, j1 = g * SG * NSLOT, (g + 1) * SG * NSLOT
                    _, vs = nc.values_load_multi_w_load_instructions(
                        gb_s[0:1, j0:j1:2],
                        engines=[mybir.EngineType.SP],
                        skip_runtime_bounds_check=True)
                    vals_sp.extend(vs)
                    _, va = nc.values_load_multi_w_load_instructions(
                        gb_s[0:1, j0 + 1:j1:2],
                        engines=[mybir.EngineType.Activation],
                        skip_runtime_bounds_check=True)
                    vals_act.extend(va)

                chi4 = p3c.tile([4, 2, SLOC, 4, B], BF16)
                chirow = ps_r.tile([1, NINST * B], F32)

                for g in range(NG):
                    X8 = p3x8.tile([128, SG, NSLOT, 2, B], F8, tag="X8")
                    j0 = g * SG * NSLOT
                    for sl in range(SG):
                        for k in range(NSLOT):
                            j = j0 + sl * NSLOT + k
                            if j % 2 == 0:
                                eng, v = nc.sync, vals_sp[j // 2]
                            else:
                                eng, v = nc.scalar, vals_act[j // 2]
                            eng.dma_start(
                                out=X8[:, sl, k, :, :],
                                in_=qv_flat[bass.ds(v, 128), :])

                    if g == 0:
                        # PE re-warm on the first gathered data: runs in
                        # the tail of the collective gap, right before
                        # the real phase-3 matmuls
                        wp2 = psw.tile([64, 2 * B], F32)
                        for k in range(16):
                            nc.tensor.matmul(
                                wp2[:], ident_s[:, 0:64],
                                X8[:, 0, 0, :, :],
                                start=(k == 0), stop=(k == 15))

                    # upconvert fp8 -> bf16 on ACT (keeps DVE min at 2x)
                    X = p3.tile([128, SG, NSLOT, 2, B], BF16, tag="X")
                    for sl in range(SG):
                        nc.scalar.activation(
                            X[:, sl], X8[:, sl],
                            mybir.ActivationFunctionType.Copy)

                    PM = p3pm.tile([128, SG, NPAIR, 2, B], BF16, tag="PM")
                    off = 0
                    for dd in range(1, K):
                        n = K - dd
                        nc.vector.tensor_tensor(
                            PM[:, :, off:off + n, :, :],
                            X[:, :, 1:1 + n, :, :],
                            X[:, :, 1 + dd:1 + dd + n, :, :],
                            mybir.AluOpType.min)
                        off += n

                    for sl in range(SG):
                        s = g * SG + sl
                        for t in range(2):
                            P = ps_p.tile([4, 4 * B], F32, tag="P")
                            for j in range(7):
                                nc.tensor.matmul(
                                    P[:], pw_s[:, s, j, :],
                                    PM[:, sl, 4 * j:4 * j + 4, t, :],
                                    start=(j == 0), stop=False)
                            # singles + center: off-diag pollution is
                            # discarded by the fold
                            nc.tensor.matmul(
                                P[:], ws_s[:, s, :], sq_rep[:, t, :, :],
                                start=False, stop=True)
                            nc.scalar.copy(
                                chi4[:, t, s, :, :].rearrange(
                                    "p a b -> p (a b)"), P[:])

                    # fold half as soon as its 4 sets are done
                    if g % 2 == 1:
                        hh = g // 2
                        for t in range(2):
                            dst = chirow[:, t * SLOC * B + hh * 4 * B:
                                         t * SLOC * B + (hh + 1) * 4 * B]
                            for c in range(4):
                                nc.tensor.matmul(
                                    dst, ident_s[0:4, c:c + 1],
                                    chi4[:, t, 4 * hh:4 * hh + 4, c, :],
                                    start=(c == 0), stop=(c == 3))

                # copy + store each folded half as soon as it is ready
                chirow_s = p3c.tile([1, NINST * B], F32)
                for t in range(2):
                    for hh in range(2):
                        lo = t * SLOC * B + hh * 4 * B
                        hi2 = t * SLOC * B + (hh + 1) * 4 * B
                        nc.scalar.copy(chirow_s[:, lo:hi2],
                                       chirow[:, lo:hi2])
                        nc.sync.dma_start(out=chi[:, lo:hi2],
                                          in_=chirow_s[:, lo:hi2])

    nc.compile()
    return nc


def _prepare_inputs(observation, action, local_edges, V_W1, V_b1, V_g1,
                    V_beta1, V_W2, V_b2, A_W1, A_b1, A_g1, A_beta1, A_W2,
                    A_b2, chi_m1, chi_m2):
    bf16 = ml_dtypes.bfloat16
    centers = np.asarray(local_edges[:, 0, 0]).astype(np.int64)
    neigh = np.asarray(local_edges[:, 0, 1:]).astype(np.int64)
    m1s = chi_m1.sum(1) / (HEADS * D)              # [S, K]
    tri = np.triu(np.ones((K, K), np.float32), k=1)
    m2s = (chi_m2.sum(1) * tri) / (HEADS * D)      # [S, K, K]

    lnw = np.zeros((128, 8), np.float32)
    lnw[:, 0] = V_g1[:128];    lnw[:, 1] = V_g1[128:]
    lnw[:, 2] = V_beta1[:128]; lnw[:, 3] = V_beta1[128:]
    lnw[:, 4] = A_g1[:128];    lnw[:, 5] = A_g1[128:]
    lnw[:, 6] = A_beta1[:128]; lnw[:, 7] = A_beta1[128:]

    in_maps = []
    for c in range(NCORE):
        nodes = slice(c * NLOC, (c + 1) * NLOC)
        m = {}
        obs_n = observation[:, nodes, :].transpose(1, 2, 0)  # [8, H, B]
        act_n = action[:, nodes, :].transpose(1, 2, 0)
        xin = np.concatenate(
            [obs_n.reshape(NLOC, 2, 128, B).transpose(0, 2, 1, 3),
             act_n.reshape(NLOC, 2, 128, B).transpose(0, 2, 1, 3)],
            axis=2)                                           # [8,128,4,B]
        m["xin"] = np.ascontiguousarray(xin).astype(bf16)

        w1v = V_W1[nodes].reshape(NLOC, 2, 128, H).transpose(0, 2, 1, 3)
        w1a = A_W1[nodes].reshape(NLOC, 4, 128, H).transpose(0, 2, 1, 3)
        w2v = V_W2[nodes].reshape(NLOC, 2, 128, D).transpose(0, 2, 1, 3)
        w2a = A_W2[nodes].reshape(NLOC, 2, 128, D).transpose(0, 2, 1, 3)
        m["wp"] = np.ascontiguousarray(np.concatenate(
            [w1v.reshape(NLOC, 128, 512),
             w1a.reshape(NLOC, 128, 1024),
             w2v.reshape(NLOC, 128, 256),
             w2a.reshape(NLOC, 128, 256)], axis=2)).astype(bf16)
        m["bia"] = np.ascontiguousarray(np.concatenate(
            [V_b1[nodes], A_b1[nodes]],
            axis=1).reshape(1, NLOC * 512)).astype(bf16)
        b2 = np.stack([V_b2[nodes], A_b2[nodes]], axis=2)  # [8, 128, 2]
        m["b2p"] = np.ascontiguousarray(
            b2.transpose(1, 0, 2)).astype(np.float32)
        m["lnw"] = lnw

        pwn = np.zeros((SLOC, 7, 4), np.float32)
        wsn = np.zeros((64, SLOC, 4), np.float32)
        gbn = np.zeros((1, SLOC * NSLOT), np.int32)
        for sl in range(SLOC):
            s = c * SLOC + sl
            for p, (a, b_) in enumerate(PAIRS):
                pwn[sl, p // 4, p % 4] = m2s[s, a - 1, b_ - 1]
            w = np.zeros(64, np.float32)
            for k in range(K):
                w[neigh[s, k]] += m1s[s, k]
            w[centers[s]] += 1.0 / D
            # each of the 4 fold-diagonal rows carries 1/4 of the singles
            wsn[:, sl, :] = w[:, None] / 4.0
            slots = [int(centers[s])] + [int(x) for x in neigh[s]]
            for k in range(NSLOT):
                g = slots[k]
                hh, cc, i4 = (g % NLOC) // 4, g // NLOC, (g % NLOC) % 4
                gbn[0, sl * NSLOT + k] = ((cc * 2 + hh) * CH_ROWS
                                          + i4 * 128)
        m["pw"] = np.broadcast_to(
            pwn.astype(bf16)[None], (128, SLOC, 7, 4)).copy()
        m["ws"] = wsn.astype(bf16)
        m["gb"] = gbn
        m["ident"] = np.eye(128, dtype=np.float32).astype(bf16)
        in_maps.append(m)
    return in_maps


def kernel(**inputs):
    global _compiled
    if _compiled is None:
        _compiled = _build()
    nc = _compiled
    inputs = {k: np.asarray(v) for k, v in inputs.items()}
    in_maps = _prepare_inputs(**inputs)
    res = run_bass_kernel_spmd(nc, in_maps, list(range(NCORE)))
    global _last_results
    _last_results = res
    chi_q = np.zeros((B, N), np.float32)
    chi_v = np.zeros((B, N), np.float32)
    for c in range(NCORE):
        out = res.results[c]["chi"].reshape(2, SLOC, B)
        for sl in range(SLOC):
            chi_q[:, c * SLOC + sl] = out[0, sl]
            chi_v[:, c * SLOC + sl] = out[1, sl]
    return chi_q, chi_v


# revision 33
# speedup vs baseline: 1.1529x; 1.1529x over previous
"""Trainium2 Bass kernel for nn_Critic (gnn_message_passing) — v4.

Strategy (8 NeuronCores, one SPMD NEFF):
  Phase 1 (node-sharded, 8 nodes/core): per-node MLPs fully in bf16 on PE.
    LN rsqrt = ACT Sqrt(var+eps) + DVE reciprocal; scale/bias/relu fused
    into one ACT Relu(g*x+be) per chunk (every LUT function used lives in
    the single 'sqrt_and_others' table -> one table load total). The mm2
    biases ride for free on the existing ACT copy (bias=b2V) and the DVE
    q=A+V op (scalar_tensor_tensor +b2A). Per node we emit the d-sums of
    Q and V (one ones-matmul) — the Choquet singles+center source — in
    bf16, and an fp8 copy of Q,V for the collective.
  Phase 2: AllGather of the fp8 Q/V block (half the bytes of bf16), then
    a second tiny AllGather of the bf16 d-sums. fp8 only touches the pair
    (min) terms whose error is ~10x below the center term carried by the
    bf16 sums.
  Phase 3 (set-sharded): X gathered as fp8 64-row blocks via
    register-offset HWDGE DMAs (offsets preloaded on both queues during
    the collective), upconverted fp8->bf16 on the ACT engine so the DVE
    pairmin keeps its 2x 16-bit rate; weighted d+pair reduction on PE via
    diagonal-block matmuls [4, 4B]; singles+center ride as one extra
    accumulated matmul per (set,t) whose off-diagonal garbage lands in
    fold-discarded entries; fold + output stream out per half.
"""

import numpy as np
import ml_dtypes

import concourse.bass as bass
import concourse.bacc as bacc
import concourse.mybir as mybir
from concourse import tile
from concourse.bass_utils import run_bass_kernel_spmd

B, N, H, D, K, HEADS = 128, 64, 256, 128, 8, 3
NCORE = 8
NLOC = N // NCORE      # nodes per core
SLOC = N // NCORE      # sets per core
NINST = 2 * SLOC       # (set, t) instances per core
NSLOT = K + 1          # center + 8 neighbors
NPAIR = (K * (K - 1)) // 2  # 28
CH_ROWS = 4 * D + 16  # rows per fp8 chunk: 4 nodes + 16 sums-byte rows
F32 = mybir.dt.float32
BF16 = mybir.dt.bfloat16
F8 = mybir.dt.float8e4
I32 = mybir.dt.int32

# pairs in delta-major order over neighbor slots 1..8
PAIRS = [(a, a + d) for d in range(1, K) for a in range(1, K - d + 1)]

_compiled = None


def _build():
    nc = bacc.Bacc("TRN2", target_bir_lowering=False, debug=False,
                   num_devices=NCORE)

    # ---- per-core inputs (host-packed) ----
    xin = nc.dram_tensor("xin", [NLOC, 128, 4, B], BF16, kind="ExternalInput")
    wp = nc.dram_tensor("wp", [NLOC, 128, 2048], BF16, kind="ExternalInput")
    bia = nc.dram_tensor("bia", [1, NLOC * 512], BF16, kind="ExternalInput")
    b2p = nc.dram_tensor("b2p", [128, NLOC, 2], F32, kind="ExternalInput")
    lnw = nc.dram_tensor("lnw", [128, 8], F32, kind="ExternalInput")
    pw = nc.dram_tensor("pw", [128, SLOC, 7, 4], BF16, kind="ExternalInput")
    ws = nc.dram_tensor("ws", [64, SLOC, 4], BF16, kind="ExternalInput")
    gb = nc.dram_tensor("gb", [1, SLOC * NSLOT], I32, kind="ExternalInput")
    ident = nc.dram_tensor("ident", [128, 128], BF16, kind="ExternalInput")

    chi = nc.dram_tensor("chi", [1, NINST * B], F32, kind="ExternalOutput")

    with tile.TileContext(nc, num_cores=NCORE) as tc:
        with tc.tile_pool(name="const", bufs=1) as cpool, \
             tc.tile_pool(name="dram", bufs=1, space="DRAM") as dram:
            ident_s = cpool.tile([128, 128], BF16)
            nc.sync.dma_start(out=ident_s[:], in_=ident[:])
            ones_row = cpool.tile([1, 128], BF16)
            nc.vector.memset(ones_row[:], 1.0)
            ones_col = cpool.tile([128, 1], BF16)
            nc.vector.memset(ones_col[:], 1.0)
            eps_t = cpool.tile([B, 1], F32)
            nc.vector.memset(eps_t[:], 1e-5)
            lnw_s = cpool.tile([128, 8], F32)
            nc.sync.dma_start(out=lnw_s[:], in_=lnw[:])
            b2p_s = cpool.tile([128, NLOC, 2], F32)
            nc.sync.dma_start(out=b2p_s[:], in_=b2p[:])
            pw_s = cpool.tile([128, SLOC, 7, 4], BF16)
            nc.scalar.dma_start(out=pw_s[:], in_=pw[:])
            ws_s = cpool.tile([64, SLOC, 4], BF16)
            nc.scalar.dma_start(out=ws_s[:], in_=ws[:])
            bia_s = cpool.tile([1, NLOC * 512], BF16)
            nc.gpsimd.dma_start(out=bia_s[:], in_=bia[:])
            gb_s = cpool.tile([1, SLOC * NSLOT], I32)
            nc.sync.dma_start(out=gb_s[:], in_=gb[:])
            warm_rhs = cpool.tile([128, 512], BF16)
            nc.vector.memset(warm_rhs[:], 0.0)

            # fp8 gathered q/v: [core, chunk, row, (t b)]; the last 16
            # rows of each chunk carry the BYTES of the bf16 d-sums
            # (chunk 1) so one collective moves everything.
            qvloc8 = dram.tile([2, CH_ROWS, 2 * B], F8)
            qvall8 = dram.tile([NCORE, 2, CH_ROWS, 2 * B], F8,
                               addr_space="Shared")
            qv_flat = qvall8.rearrange("c h r w -> (c h r) w")

            # HAM warm-up: lift the PE clock gate before real matmuls.
            with tc.tile_pool(name="ps_w", bufs=1, space="PSUM") as ps_w:
                wpp = ps_w.tile([128, 512], F32)
                for k in range(10):
                    nc.tensor.matmul(wpp[:], ident_s[:], warm_rhs[:],
                                     start=(k == 0), stop=(k == 9))

            # ================= Phase 1: per-node MLPs =================
            with tc.tile_pool(name="p1", bufs=4) as p1, \
                 tc.tile_pool(name="p1w", bufs=3) as p1w, \
                 tc.tile_pool(name="p1s", bufs=1) as p1s, \
                 tc.tile_pool(name="ps_h", bufs=3, space="PSUM") as ps_h, \
                 tc.tile_pool(name="ps_t", bufs=2, space="PSUM") as ps_t, \
                 tc.tile_pool(name="ps_o", bufs=2, space="PSUM") as ps_o, \
                 tc.tile_pool(name="ps_s", bufs=1, space="PSUM") as ps_s:

                sums8 = p1s.tile([1, NLOC * 2 * B], BF16)

                for i in range(NLOC):
                    xt = p1.tile([128, 4, B], BF16, tag="xt")
                    nc.sync.dma_start(out=xt[:], in_=xin[i])
                    wt = p1w.tile([128, 2048], BF16, tag="wt")
                    nc.scalar.dma_start(out=wt[:], in_=wp[i])
                    bo = i * 512

                    # mm1 for both branches: h[b, (br, o)] in PSUM f32;
                    # both b1 biases in one 512-col matmul
                    h = ps_h.tile([B, 2, H], F32, tag="h")
                    for c in range(2):
                        nc.tensor.matmul(h[:, 0, :], xt[:, c, :],
                                         wt[:, c * 256:(c + 1) * 256],
                                         start=(c == 0), stop=False)
                    nc.tensor.matmul(h[:, 0, :], ones_row[:],
                                     bia_s[:, bo:bo + 256],
                                     start=False, stop=True)
                    for c in range(4):
                        nc.tensor.matmul(h[:, 1, :], xt[:, c, :],
                                         wt[:, 512 + c * 256:768 + c * 256],
                                         start=(c == 0), stop=False)
                    nc.tensor.matmul(h[:, 1, :], ones_row[:],
                                     bia_s[:, bo + 256:bo + 512],
                                     start=False, stop=True)

                    # LN + relu per branch, then mm2
                    ut = ps_t.tile([128, 2, 2, 128], BF16, tag="ut")
                    hT = p1.tile([128, 2, 2, 128], BF16, tag="hT")
                    o = ps_o.tile([D, 2, B], F32, tag="o")
                    for br in range(2):
                        bn6 = p1.tile([B, 6], F32, tag="bn6")
                        nc.vector.bn_stats(bn6[:], h[:, br, :])
                        bn2 = p1.tile([B, 2], F32, tag="bn2")
                        nc.vector.bn_aggr(bn2[:], bn6[:])
                        sd = p1.tile([B, 1], F32, tag="sd")
                        nc.scalar.activation(
                            sd[:], bn2[:, 1:2],
                            mybir.ActivationFunctionType.Sqrt,
                            bias=eps_t[:])
                        rs = p1.tile([B, 1], F32, tag="rs")
                        nc.vector.reciprocal(rs[:], sd[:])
                        # u = (h - mu) * rs
                        u = p1.tile([B, H], BF16, tag="u")
                        nc.vector.tensor_scalar(u[:], h[:, br, :],
                                                bn2[:, 0:1], rs[:],
                                                mybir.AluOpType.subtract,
                                                mybir.AluOpType.mult)
                        for c in range(2):
                            nc.tensor.transpose(ut[:, br, c, :],
                                                u[:, c * 128:(c + 1) * 128],
                                                ident_s[:])
                            # hT = relu(g * ut + be), one ACT op
                            nc.scalar.activation(
                                hT[:, br, c, :], ut[:, br, c, :],
                                mybir.ActivationFunctionType.Relu,
                                scale=lnw_s[:, 4 * br + c:4 * br + c + 1],
                                bias=lnw_s[:, 4 * br + 2 + c:4 * br + 3 + c])
                        w2o = 1536 + br * 256
                        for c in range(2):
                            nc.tensor.matmul(
                                o[:, br, :],
                                wt[:, w2o + c * 128:w2o + (c + 1) * 128],
                                hT[:, br, c, :],
                                start=(c == 0), stop=(c == 1))

                    qvb = p1.tile([D, 2, B], BF16, tag="qvb")
                    # v = oV + b2V (ACT copy with per-partition bias)
                    nc.scalar.activation(
                        qvb[:, 1, :], o[:, 0, :],
                        mybir.ActivationFunctionType.Identity,
                        bias=b2p_s[:, i, 0:1])
                    # q = (oA + b2A) + v  (DVE scalar_tensor_tensor)
                    nc.vector.scalar_tensor_tensor(
                        qvb[:, 0, :], o[:, 1, :], b2p_s[:, i, 1:2],
                        qvb[:, 1, :],
                        mybir.AluOpType.add, mybir.AluOpType.add)
                    # fp8 copy for the collective (Pool engine)
                    qv8 = p1.tile([D, 2, B], F8, tag="qv8")
                    nc.gpsimd.tensor_copy(qv8[:], qvb[:])
                    # d-sums of q and v in one ones-matmul
                    sq = ps_s.tile([1, 2 * B], F32, tag="sq")
                    nc.tensor.matmul(sq[:], ones_col[:],
                                     qvb[:].rearrange("p t b -> p (t b)"),
                                     start=True, stop=True)
                    nc.scalar.copy(
                        sums8[:, i * 2 * B:(i + 1) * 2 * B], sq[:])
                    # store node rows into its chunk
                    hh, i4 = (0, i) if i < 4 else (1, i - 4)
                    nc.sync.dma_start(
                        out=qvloc8[hh, i4 * 128:(i4 + 1) * 128, :],
                        in_=qv8[:].rearrange("p t b -> p (t b)"))

                nc.gpsimd.dma_start(
                    out=qvloc8[1, 512:528, :].bitcast(BF16),
                    in_=sums8[:])

            # ================= Phase 2: AllGathers ====================
            nc.gpsimd.collective_compute(
                "AllGather", mybir.AluOpType.bypass,
                replica_groups=[list(range(NCORE))],
                ins=[qvloc8.opt()], outs=[qvall8.opt()],
            )

            # ================= Phase 3: Choquet =======================
            SG = 2  # sets per group
            NG = SLOC // SG
            with tc.tile_pool(name="p3x8", bufs=2) as p3x8, \
                 tc.tile_pool(name="p3", bufs=2) as p3, \
                 tc.tile_pool(name="p3pm", bufs=2) as p3pm, \
                 tc.tile_pool(name="p3c", bufs=1) as p3c, \
                 tc.tile_pool(name="ps_p", bufs=3, space="PSUM") as ps_p, \
                 tc.tile_pool(name="ps_w2", bufs=1, space="PSUM") as psw, \
                 tc.tile_pool(name="ps_r", bufs=1, space="PSUM") as ps_r:

                # d-sums for all 64 nodes -> [64, 2, B], replicated x4
                sq_all = p3c.tile([64, 2, B], BF16)
                nc.sync.dma_start(
                    out=sq_all[:],
                    in_=qvall8[:, 1, 512:528, :].bitcast(BF16))
                sq_rep = p3c.tile([64, 2, 4, B], BF16)
                for r in range(4):
                    nc.vector.tensor_copy(sq_rep[:, :, r, :], sq_all[:])

                # preload gather offsets while the collective is in
                # flight; each issuing engine holds only what it uses
                vals_sp, vals_act = [], []
                for g in range(NG):
                    j0, j1 = g * SG * NSLOT, (g + 1) * SG * NSLOT
                    _, vs = nc.values_load_multi_w_load_instructions(
                        gb_s[0:1, j0:j1:2],
                        engines=[mybir.EngineType.SP],
                        skip_runtime_bounds_check=True)
                    vals_sp.extend(vs)
                    _, va = nc.values_load_multi_w_load_instructions(
                        gb_s[0:1, j0 + 1:j1:2],
                        engines=[mybir.EngineType.Pool],
                        skip_runtime_bounds_check=True)
                    vals_act.extend(va)

                chi4 = p3c.tile([4, 2, SLOC, 4, B], BF16)
                chirow = ps_r.tile([1, NINST * B], F32)

                for g in range(NG):
                    X8 = p3x8.tile([128, SG, NSLOT, 2, B], F8, tag="X8")
                    j0 = g * SG * NSLOT
                    for sl in range(SG):
                        for k in range(NSLOT):
                            j = j0 + sl * NSLOT + k
                            if j % 2 == 0:
                                eng, v = nc.sync, vals_sp[j // 2]
                            else:
                                eng, v = nc.gpsimd, vals_act[j // 2]
                            eng.dma_start(
                                out=X8[:, sl, k, :, :],
                                in_=qv_flat[bass.ds(v, 128), :])

                    if g == 0:
                        # PE re-warm on the first gathered data: runs in
                        # the tail of the collective gap, right before
                        # the real phase-3 matmuls
                        wp2 = psw.tile([64, 2 * B], F32)
                        for k in range(16):
                            nc.tensor.matmul(
                                wp2[:], ident_s[:, 0:64],
                                X8[:, 0, 0, :, :],
                                start=(k == 0), stop=(k == 15))

                    # upconvert fp8 -> bf16 on ACT (keeps DVE min at 2x)
                    X = p3.tile([128, SG, NSLOT, 2, B], BF16, tag="X")
                    for sl in range(SG):
                        nc.scalar.activation(
                            X[:, sl], X8[:, sl],
                            mybir.ActivationFunctionType.Copy)

                    PM = p3pm.tile([128, SG, NPAIR, 2, B], BF16, tag="PM")
                    off = 0
                    for dd in range(1, K):
                        n = K - dd
                        nc.vector.tensor_tensor(
                            PM[:, :, off:off + n, :, :],
                            X[:, :, 1:1 + n, :, :],
                            X[:, :, 1 + dd:1 + dd + n, :, :],
                            mybir.AluOpType.min)
                        off += n

                    for sl in range(SG):
                        s = g * SG + sl
                        for t in range(2):
                            P = ps_p.tile([4, 4 * B], F32, tag="P")
                            for j in range(7):
                                nc.tensor.matmul(
                                    P[:], pw_s[:, s, j, :],
                                    PM[:, sl, 4 * j:4 * j + 4, t, :],
                                    start=(j == 0), stop=False)
                            # singles + center: off-diag pollution is
                            # discarded by the fold
                            nc.tensor.matmul(
                                P[:], ws_s[:, s, :], sq_rep[:, t, :, :],
                                start=False, stop=True)
                            nc.scalar.copy(
                                chi4[:, t, s, :, :].rearrange(
                                    "p a b -> p (a b)"), P[:])

                    # fold half as soon as its 4 sets are done
                    if g % 2 == 1:
                        hh = g // 2
                        for t in range(2):
                            dst = chirow[:, t * SLOC * B + hh * 4 * B:
                                         t * SLOC * B + (hh + 1) * 4 * B]
                            for c in range(4):
                                nc.tensor.matmul(
                                    dst, ident_s[0:4, c:c + 1],
                                    chi4[:, t, 4 * hh:4 * hh + 4, c, :],
                                    start=(c == 0), stop=(c == 3))

                # copy + store each folded half as soon as it is ready
                chirow_s = p3c.tile([1, NINST * B], F32)
                for t in range(2):
                    for hh in range(2):
                        lo = t * SLOC * B + hh * 4 * B
                        hi2 = t * SLOC * B + (hh + 1) * 4 * B
                        nc.scalar.copy(chirow_s[:, lo:hi2],
                                       chirow[:, lo:hi2])
                        nc.sync.dma_start(out=chi[:, lo:hi2],
                                          in_=chirow_s[:, lo:hi2])

    nc.compile()
    return nc


def _prepare_inputs(observation, action, local_edges, V_W1, V_b1, V_g1,
                    V_beta1, V_W2, V_b2, A_W1, A_b1, A_g1, A_beta1, A_W2,
                    A_b2, chi_m1, chi_m2):
    bf16 = ml_dtypes.bfloat16
    centers = np.asarray(local_edges[:, 0, 0]).astype(np.int64)
    neigh = np.asarray(local_edges[:, 0, 1:]).astype(np.int64)
    m1s = chi_m1.sum(1) / (HEADS * D)              # [S, K]
    tri = np.triu(np.ones((K, K), np.float32), k=1)
    m2s = (chi_m2.sum(1) * tri) / (HEADS * D)      # [S, K, K]

    lnw = np.zeros((128, 8), np.float32)
    lnw[:, 0] = V_g1[:128];    lnw[:, 1] = V_g1[128:]
    lnw[:, 2] = V_beta1[:128]; lnw[:, 3] = V_beta1[128:]
    lnw[:, 4] = A_g1[:128];    lnw[:, 5] = A_g1[128:]
    lnw[:, 6] = A_beta1[:128]; lnw[:, 7] = A_beta1[128:]

    in_maps = []
    for c in range(NCORE):
        nodes = slice(c * NLOC, (c + 1) * NLOC)
        m = {}
        obs_n = observation[:, nodes, :].transpose(1, 2, 0)  # [8, H, B]
        act_n = action[:, nodes, :].transpose(1, 2, 0)
        xin = np.concatenate(
            [obs_n.reshape(NLOC, 2, 128, B).transpose(0, 2, 1, 3),
             act_n.reshape(NLOC, 2, 128, B).transpose(0, 2, 1, 3)],
            axis=2)                                           # [8,128,4,B]
        m["xin"] = np.ascontiguousarray(xin).astype(bf16)

        w1v = V_W1[nodes].reshape(NLOC, 2, 128, H).transpose(0, 2, 1, 3)
        w1a = A_W1[nodes].reshape(NLOC, 4, 128, H).transpose(0, 2, 1, 3)
        w2v = V_W2[nodes].reshape(NLOC, 2, 128, D).transpose(0, 2, 1, 3)
        w2a = A_W2[nodes].reshape(NLOC, 2, 128, D).transpose(0, 2, 1, 3)
        m["wp"] = np.ascontiguousarray(np.concatenate(
            [w1v.reshape(NLOC, 128, 512),
             w1a.reshape(NLOC, 128, 1024),
             w2v.reshape(NLOC, 128, 256),
             w2a.reshape(NLOC, 128, 256)], axis=2)).astype(bf16)
        m["bia"] = np.ascontiguousarray(np.concatenate(
            [V_b1[nodes], A_b1[nodes]],
            axis=1).reshape(1, NLOC * 512)).astype(bf16)
        b2 = np.stack([V_b2[nodes], A_b2[nodes]], axis=2)  # [8, 128, 2]
        m["b2p"] = np.ascontiguousarray(
            b2.transpose(1, 0, 2)).astype(np.float32)
        m["lnw"] = lnw

        pwn = np.zeros((SLOC, 7, 4), np.float32)
        wsn = np.zeros((64, SLOC, 4), np.float32)
        gbn = np.zeros((1, SLOC * NSLOT), np.int32)
        for sl in range(SLOC):
            s = c * SLOC + sl
            for p, (a, b_) in enumerate(PAIRS):
                pwn[sl, p // 4, p % 4] = m2s[s, a - 1, b_ - 1]
            w = np.zeros(64, np.float32)
            for k in range(K):
                w[neigh[s, k]] += m1s[s, k]
            w[centers[s]] += 1.0 / D
            # each of the 4 fold-diagonal rows carries 1/4 of the singles
            wsn[:, sl, :] = w[:, None] / 4.0
            slots = [int(centers[s])] + [int(x) for x in neigh[s]]
            for k in range(NSLOT):
                g = slots[k]
                hh, cc, i4 = (g % NLOC) // 4, g // NLOC, (g % NLOC) % 4
                gbn[0, sl * NSLOT + k] = ((cc * 2 + hh) * CH_ROWS
                                          + i4 * 128)
        m["pw"] = np.broadcast_to(
            pwn.astype(bf16)[None], (128, SLOC, 7, 4)).copy()
        m["ws"] = wsn.astype(bf16)
        m["gb"] = gbn
        m["ident"] = np.eye(128, dtype=np.float32).astype(bf16)
        in_maps.append(m)
    return in_maps


def kernel(**inputs):
    global _compiled
    if _compiled is None:
        _compiled = _build()
    nc = _compiled
    inputs = {k: np.asarray(v) for k, v in inputs.items()}
    in_maps = _prepare_inputs(**inputs)
    res = run_bass_kernel_spmd(nc, in_maps, list(range(NCORE)))
    global _last_results
    _last_results = res
    chi_q = np.zeros((B, N), np.float32)
    chi_v = np.zeros((B, N), np.float32)
    for c in range(NCORE):
        out = res.results[c]["chi"].reshape(2, SLOC, B)
        for sl in range(SLOC):
            chi_q[:, c * SLOC + sl] = out[0, sl]
            chi_v[:, c * SLOC + sl] = out[1, sl]
    return chi_q, chi_v
